# revision 13
# baseline (speedup 1.0000x reference)
"""GQA kernel for 8 TRN2 NeuronCores.

Model: B=4, T=2048, C=1024, 16 q heads / 4 kv heads / head_dim 64, causal.
Sharding: 16 (batch, kv-head-pair) units -> 2 per core. Core c handles batch
c//2 and kv-head pair (0,1) if c even else (2,3), i.e. q heads 0-7 or 8-15.
Each core computes its slice of the Q/K/V projections, local causal
attention, and a partial output projection (its 512 columns of the head
concat); the host sums the two partial y's per batch.

v2: all matmuls run in float32r (TF32-style, 1 cycle/row vs 4 for fp32, so
4x PE throughput; inputs are rounded to ~10 mantissa bits which costs
~1e-4 relative error). The BIR verifier requires every matmul operand to be
produced as float32r, so projection weights / x stream in as fp32r-typed
DRAM tensors and on-chip producers (DVE copies, the exp activation, the
normalize muls) write fp32r tiles directly.

Fused software pipeline over token blocks of 512: A(t) projections for
token block t -> B(c=t) attention for query block c (needs only k/v token
blocks <= c by causality) -> C(c) output projection rows for block c.

Attention per (pair p, query block c): head pair shares one qT tile
([128, T]: rows 0-63 head A, 64-127 head B). Scores for A and B are two
concurrent row-tiled K=64 matmuls into one 2-bank PSUM tile; one Exp
activation covers both; causal masking via one gpsimd affine_select per
diagonal block (which also zero-fills the columns the restricted
exp skipped). PV uses Vext=[V|1] (M=65) so the softmax denominator falls
out of row 64 of the PV accumulator; reciprocal via the fast approx DVE op
(~51 ULP) after a [1,1024] copy to SBUF (custom DVE ops can't read PSUM),
then gpsimd partition_broadcast (dst must start at partition 0).
"""

import numpy as np

T = 2048
C = 1024
HD = 64
P = 128
TQ = 512
NTQ = T // TQ  # 4
NTK = T // P   # 16
QCOLS = 512

_PROG = None


def _build_program():
    import concourse.mybir as mybir
    import concourse.tile as tile
    from concourse import bacc

    FP32 = mybir.dt.float32
    FP32R = mybir.dt.float32r
    AF = mybir.ActivationFunctionType
    ALU = mybir.AluOpType

    nc = bacc.Bacc("TRN2", target_bir_lowering=False, debug=False, num_devices=8)

    xT = nc.dram_tensor("xT", [C, T], FP32R, kind="ExternalInput").ap()
    wqT = nc.dram_tensor("wqT", [C, QCOLS], FP32R, kind="ExternalInput").ap()
    wkT = nc.dram_tensor("wkT", [C, 128], FP32R, kind="ExternalInput").ap()
    wvT = nc.dram_tensor("wvT", [C, 128], FP32R, kind="ExternalInput").ap()
    woT = nc.dram_tensor("woT", [QCOLS, C], FP32R, kind="ExternalInput").ap()
    ones = nc.dram_tensor("ones", [P, NTK], FP32R, kind="ExternalInput").ap()
    y = nc.dram_tensor("y", [T, C], FP32, kind="ExternalOutput").ap()

    with tile.TileContext(nc) as tc:
        with tc.tile_pool(name="const", bufs=1) as cpool, \
             tc.tile_pool(name="persist", bufs=1) as pp:
            # ident2: two stacked 64x64 identity blocks, so a transpose whose
            # input lives at base partition 64 can use ident2[64:128, :]
            ident2 = cpool.tile([P, 64], FP32, tag="ident2")
            nc.gpsimd.memset(ident2[:], 0.0)
            for blk in range(2):
                nc.gpsimd.affine_select(
                    out=ident2[:], in_=ident2[:],
                    compare_op=ALU.not_equal, fill=1.0,
                    base=-64 * blk, pattern=[[-1, 64]], channel_multiplier=1,
                )

            kT = pp.tile([P, T], FP32R, tag="kT")
            # vext[kv][:, j] = [V_kv tile j | 1]  [128, 65] slices
            vextA = pp.tile([P, NTK, HD + 1], FP32R, tag="veA")
            vextB = pp.tile([P, NTK, HD + 1], FP32R, tag="veB")

            wq = [pp.tile([P, QCOLS], FP32R, tag=f"wq{j}", name=f"wq{j}") for j in range(8)]
            wk = [pp.tile([P, 128], FP32R, tag=f"wk{j}", name=f"wk{j}") for j in range(8)]
            wv = [pp.tile([P, 128], FP32R, tag=f"wv{j}", name=f"wv{j}") for j in range(8)]
            wo = [pp.tile([P, C], FP32R, tag=f"wo{p}", name=f"wo{p}") for p in range(4)]
            for j in range(8):
                nc.sync.dma_start(out=wq[j][:], in_=wqT[P * j:P * (j + 1), :])
                nc.sync.dma_start(out=wk[j][:], in_=wkT[P * j:P * (j + 1), :])
                nc.sync.dma_start(out=wv[j][:], in_=wvT[P * j:P * (j + 1), :])
            for p in range(4):
                nc.sync.dma_start(out=wo[p][:], in_=woT[P * p:P * (p + 1), :])
            # the softmax-denominator ones columns, written once
            nc.sync.dma_start(out=vextA[:, :, HD], in_=ones[:])
            nc.sync.dma_start(out=vextB[:, :, HD], in_=ones[:])

            with tc.tile_pool(name="xw", bufs=2) as xw, \
                 tc.tile_pool(name="prps", bufs=2, space="PSUM") as prps, \
                 tc.tile_pool(name="scps", bufs=2, space="PSUM") as scps, \
                 tc.tile_pool(name="otps", bufs=1, space="PSUM") as otps, \
                 tc.tile_pool(name="ptsb", bufs=4) as ptsb, \
                 tc.tile_pool(name="blk", bufs=2) as blk, \
                 tc.tile_pool(name="nrm", bufs=1) as nrm:
                for t in range(NTQ):
                    ts = slice(TQ * t, TQ * (t + 1))
                    # ---------- stage A(t): projections for token block t
                    xt = xw.tile([P, 8, TQ], FP32R, tag="xt")
                    nc.sync.dma_start(out=xt[:], in_=xT[:, ts].rearrange("(k p) q -> p k q", p=P))
                    qTs = [blk.tile([P, TQ], FP32R, tag=f"q{f}", name=f"qt{f}")
                           for f in range(4)]
                    for f in range(4):
                        ps = prps.tile([P, TQ], FP32, tag="pa")
                        for k in range(8):
                            nc.tensor.matmul(
                                ps[:],
                                wq[k][:, P * f:P * (f + 1)],
                                xt[:, k, :],
                                start=(k == 0), stop=(k == 7),
                            )
                        nc.vector.tensor_copy(qTs[f][:], ps[:])
                    ps = prps.tile([P, TQ], FP32, tag="pa")
                    for k in range(8):
                        nc.tensor.matmul(ps[:], wk[k][:], xt[:, k, :],
                                         start=(k == 0), stop=(k == 7))
                    nc.vector.tensor_copy(kT[:, ts], ps[:])
                    vT = blk.tile([P, TQ], FP32, tag="vT")
                    ps = prps.tile([P, TQ], FP32, tag="pa")
                    for k in range(8):
                        nc.tensor.matmul(ps[:], wv[k][:], xt[:, k, :],
                                         start=(k == 0), stop=(k == 7))
                    nc.vector.tensor_copy(vT[:], ps[:])

                    # V to token-major via PE transpose: [64, 128] -> [128, 64]
                    for kv in range(2):
                        ve = vextA if kv == 0 else vextB
                        for jj in range(4):
                            j = 4 * t + jj
                            tp = prps.tile([P, TQ], FP32, tag="pa")
                            nc.tensor.transpose(
                                tp[:, 0:HD],
                                vT[64 * kv:64 * kv + 64, P * jj:P * (jj + 1)],
                                ident2[64 * kv:64 * kv + 64, :],
                            )
                            nc.vector.tensor_copy(ve[:, j, 0:HD], tp[:, 0:HD])

                    # ---------- stage B(c=t): attention for query block c
                    c = t
                    jmax = 4 * c + 3
                    attnT = [blk.tile([P, TQ], FP32R, tag=f"a{p}", name=f"at{p}")
                             for p in range(4)]
                    for p in range(4):
                        outA = otps.tile([HD + 1, TQ], FP32, tag="oA")
                        outB = otps.tile([HD + 1, TQ], FP32, tag="oB")
                        for j in range(jmax + 1):
                            r = j - 4 * c
                            # col restriction: cols < 128r are fully masked.
                            # matmul keeps N>=256 (fp32r full-rate floor)
                            moff = 128 * r if r in (1, 2) else 0
                            aoff = 128 * r if r >= 1 else 0
                            s2 = scps.tile([P, 2, TQ], FP32, tag="s2")
                            nc.tensor.matmul(
                                s2[:, 0, moff:],
                                kT[0:64, P * j:P * (j + 1)],
                                qTs[p][0:64, moff:],
                                start=True, stop=True, tile_position=(0, 0),
                            )
                            nc.tensor.matmul(
                                s2[:, 1, moff:],
                                kT[64:128, P * j:P * (j + 1)],
                                qTs[p][64:128, moff:],
                                start=True, stop=True, tile_position=(64, 0),
                            )
                            pt = ptsb.tile([P, 2, TQ], FP32R, tag="pt")
                            nc.scalar.activation(pt[:, :, aoff:], s2[:, :, aoff:],
                                                 AF.Exp, scale=0.125)
                            if r >= 0:
                                # causal triangle of the diagonal 128x128
                                # sub-block: keep where tq_local - tk >= 0
                                nc.gpsimd.affine_select(
                                    out=pt[:, :, aoff:aoff + 128],
                                    in_=pt[:, :, aoff:aoff + 128],
                                    compare_op=ALU.is_ge, fill=0.0,
                                    base=0,
                                    pattern=[[0, 2], [1, 128]],
                                    channel_multiplier=-1,
                                )
                            # cols < aoff are fully masked: never computed,
                            # never read -- PV accumulates only [aoff:] (j=0
                            # is always a full block, so PSUM is fully
                            # initialized at start)
                            nc.tensor.matmul(outA[:, aoff:], vextA[:, j, :],
                                             pt[:, 0, aoff:],
                                             start=(j == 0), stop=(j == jmax))
                            nc.tensor.matmul(outB[:, aoff:], vextB[:, j, :],
                                             pt[:, 1, aoff:],
                                             start=(j == 0), stop=(j == jmax))
                        # normalize: recip the denominator row pair, broadcast
                        # over the pair's partitions, scale PV outs into attnT
                        da = nrm.tile([1, TQ], FP32, tag="da")
                        db = nrm.tile([1, TQ], FP32, tag="db")
                        nc.vector.tensor_copy(da[:], outA[64:65, :])
                        nc.vector.tensor_copy(db[:], outB[64:65, :])
                        dar = nrm.tile([1, TQ], FP32, tag="dar")
                        dbr = nrm.tile([1, TQ], FP32, tag="dbr")
                        nc.vector.reciprocal_approx_fast(dar[:], da[:])
                        nc.vector.reciprocal_approx_fast(dbr[:], db[:])
                        bcA = nrm.tile([64, TQ], FP32, tag="bcA")
                        bcB = nrm.tile([64, TQ], FP32, tag="bcB")
                        nc.gpsimd.partition_broadcast(bcA[:], dar[:])
                        nc.gpsimd.partition_broadcast(bcB[:], dbr[:])
                        nc.vector.tensor_mul(attnT[p][0:64, :], outA[0:64, :], bcA[:])
                        nc.vector.tensor_mul(attnT[p][64:128, :], outB[0:64, :], bcB[:])

                    # ---------- stage C(c): output projection rows for block c
                    for tt in range(4 * c, 4 * c + 4):
                        jj = tt - 4 * c
                        yc = scps.tile([P, 2, TQ], FP32, tag="s2")
                        for co in range(2):
                            for p in range(4):
                                nc.tensor.matmul(
                                    yc[:, co, :],
                                    attnT[p][:, P * jj:P * (jj + 1)],
                                    wo[p][:, TQ * co:TQ * (co + 1)],
                                    start=(p == 0), stop=(p == 3),
                                )
                        yt = ptsb.tile([P, 2, TQ], FP32, tag="yt", bufs=2)
                        nc.vector.tensor_copy(yt[:], yc[:])
                        nc.sync.dma_start(out=y[P * tt:P * (tt + 1), :], in_=yt[:])

    nc.compile()
    return nc


def get_program():
    global _PROG
    if _PROG is None:
        _PROG = _build_program()
    return _PROG


def make_in_maps(x, Wq, Wk, Wv, Wo):
    """Build the per-core input dicts (host-side sharding + layout prep)."""
    x = np.asarray(x, np.float32)
    Wq = np.asarray(Wq, np.float32)
    Wk = np.asarray(Wk, np.float32)
    Wv = np.asarray(Wv, np.float32)
    Wo = np.asarray(Wo, np.float32)
    ones = np.ones((P, NTK), np.float32)
    in_maps = []
    for core in range(8):
        b, half = core // 2, core % 2
        h0 = 8 * half
        kv0 = 2 * half
        # pair-permuted local head order: [h0, h0+4, h0+1, h0+5, ...]
        heads = []
        for p in range(4):
            heads += [h0 + p, h0 + p + 4]
        qrows = np.concatenate([Wq[h * HD:(h + 1) * HD] for h in heads], 0)  # [512, C]
        wocols = np.concatenate([Wo[:, h * HD:(h + 1) * HD] for h in heads], 1)  # [C, 512]
        in_maps.append({
            "xT": np.ascontiguousarray(x[b].T),
            "wqT": np.ascontiguousarray(qrows.T),
            "wkT": np.ascontiguousarray(Wk[kv0 * HD:(kv0 + 2) * HD].T),
            "wvT": np.ascontiguousarray(Wv[kv0 * HD:(kv0 + 2) * HD].T),
            "woT": np.ascontiguousarray(wocols.T),
            "ones": ones,
        })
    return in_maps


def run_on_hw(in_maps, trace=False, **kw):
    from concourse.bass_utils import run_bass_kernel_spmd
    nc = get_program()
    return run_bass_kernel_spmd(nc, in_maps, list(range(8)), trace=trace, **kw)


def kernel(**inputs):
    in_maps = make_in_maps(
        inputs["x"], inputs["Wq"], inputs["Wk"], inputs["Wv"], inputs["Wo"]
    )
    res = run_on_hw(in_maps)
    out = np.empty((4, T, C), np.float32)
    for b in range(4):
        out[b] = res.results[2 * b]["y"] + res.results[2 * b + 1]["y"]
    return out


# revision 20
# speedup vs baseline: 1.1748x; 1.1748x over previous
"""GQA kernel for 8 TRN2 NeuronCores.

Model: B=4, T=2048, C=1024, 16 q heads / 4 kv heads / head_dim 64, causal.
Sharding: 16 (batch, kv-head-pair) units -> 2 per core. Core c handles batch
c//2 and kv-head pair (0,1) if c even else (2,3), i.e. q heads 0-7 or 8-15.
Each core computes its slice of the Q/K/V projections, local causal
attention, and a partial output projection (its 512 columns of the head
concat); the host sums the two partial y's per batch.

v3: all matmuls run on bf16 operands (1 cycle/row; fp32r measured as 2
half-matmuls = 2 cycles/row on HW, plain fp32 is 4) with fp32 PSUM
accumulation. Inputs stream in as bf16 DRAM tensors; on-chip producers
(DVE copies, the exp activation, the normalize muls) write bf16 tiles.
Costs ~0.3-1% relative error against the 2e-2 gate.

Fused software pipeline over token blocks of 512: A(t) projections for
token block t -> B(c=t) attention for query block c (needs only k/v token
blocks <= c by causality) -> C(c) output projection rows for block c.

Attention per (pair p, query block c): head pair shares one qT tile
([128, T]: rows 0-63 head A, 64-127 head B). Scores for A and B are two
concurrent row-tiled K=64 matmuls into one 2-bank PSUM tile; one Exp
activation covers both; causal masking via one gpsimd affine_select per
diagonal block (which also zero-fills the columns the restricted
exp skipped). PV uses Vext=[V|1] (M=65) so the softmax denominator falls
out of row 64 of the PV accumulator; reciprocal via the fast approx DVE op
(~51 ULP) after a [1,1024] copy to SBUF (custom DVE ops can't read PSUM),
then gpsimd partition_broadcast (dst must start at partition 0).
"""

import numpy as np

T = 2048
C = 1024
HD = 64
P = 128
TQ = 512
NTQ = T // TQ  # 4
NTK = T // P   # 16
QCOLS = 512

_PROG = None


def _build_program():
    import concourse.mybir as mybir
    import concourse.tile as tile
    from concourse import bacc

    FP32 = mybir.dt.float32
    BF16 = mybir.dt.bfloat16
    AF = mybir.ActivationFunctionType
    ALU = mybir.AluOpType

    nc = bacc.Bacc("TRN2", target_bir_lowering=False, debug=False, num_devices=8)

    xT = nc.dram_tensor("xT", [C, T], BF16, kind="ExternalInput").ap()
    wqT = nc.dram_tensor("wqT", [C, QCOLS], BF16, kind="ExternalInput").ap()
    wkT = nc.dram_tensor("wkT", [C, 128], BF16, kind="ExternalInput").ap()
    wvT = nc.dram_tensor("wvT", [C, 128], BF16, kind="ExternalInput").ap()
    woT = nc.dram_tensor("woT", [QCOLS, C], BF16, kind="ExternalInput").ap()
    ones = nc.dram_tensor("ones", [P, NTK], BF16, kind="ExternalInput").ap()
    y = nc.dram_tensor("y", [T, C], FP32, kind="ExternalOutput").ap()

    with tile.TileContext(nc) as tc:
        with tc.tile_pool(name="const", bufs=1) as cpool, \
             tc.tile_pool(name="persist", bufs=1) as pp:
            # ident2: two stacked 64x64 identity blocks, so a transpose whose
            # input lives at base partition 64 can use ident2[64:128, :]
            ident2 = cpool.tile([P, 64], FP32, tag="ident2")
            nc.gpsimd.memset(ident2[:], 0.0)
            for blk in range(2):
                nc.gpsimd.affine_select(
                    out=ident2[:], in_=ident2[:],
                    compare_op=ALU.not_equal, fill=1.0,
                    base=-64 * blk, pattern=[[-1, 64]], channel_multiplier=1,
                )

            kT = pp.tile([P, T], BF16, tag="kT")
            # vext[kv][:, j] = [V_kv tile j | 1]  [128, 65] slices
            vextA = pp.tile([P, NTK, HD + 1], BF16, tag="veA")
            vextB = pp.tile([P, NTK, HD + 1], BF16, tag="veB")

            wq = [pp.tile([P, QCOLS], BF16, tag=f"wq{j}", name=f"wq{j}") for j in range(8)]
            wk = [pp.tile([P, 128], BF16, tag=f"wk{j}", name=f"wk{j}") for j in range(8)]
            wv = [pp.tile([P, 128], BF16, tag=f"wv{j}", name=f"wv{j}") for j in range(8)]
            wo = [pp.tile([P, C], BF16, tag=f"wo{p}", name=f"wo{p}") for p in range(4)]
            for j in range(8):
                nc.sync.dma_start(out=wq[j][:], in_=wqT[P * j:P * (j + 1), :])
                nc.sync.dma_start(out=wk[j][:], in_=wkT[P * j:P * (j + 1), :])
                nc.sync.dma_start(out=wv[j][:], in_=wvT[P * j:P * (j + 1), :])
            for p in range(4):
                nc.sync.dma_start(out=wo[p][:], in_=woT[P * p:P * (p + 1), :])
            # the softmax-denominator ones columns, written once
            nc.sync.dma_start(out=vextA[:, :, HD], in_=ones[:])
            nc.sync.dma_start(out=vextB[:, :, HD], in_=ones[:])

            with tc.tile_pool(name="xw", bufs=2) as xw, \
                 tc.tile_pool(name="pp4", bufs=4, space="PSUM") as pp4, \
                 tc.tile_pool(name="scps", bufs=2, space="PSUM") as scps, \
                 tc.tile_pool(name="ptsb", bufs=6) as ptsb, \
                 tc.tile_pool(name="blk", bufs=2) as blk, \
                 tc.tile_pool(name="nrm", bufs=2) as nrm:
                for t in range(NTQ):
                    ts = slice(TQ * t, TQ * (t + 1))
                    # ---------- stage A(t): projections for token block t
                    xt = xw.tile([P, 8, TQ], BF16, tag="xt")
                    nc.sync.dma_start(out=xt[:], in_=xT[:, ts].rearrange("(k p) q -> p k q", p=P))
                    qTs = [blk.tile([P, TQ], BF16, tag=f"q{f}", name=f"qt{f}")
                           for f in range(4)]
                    for f in range(4):
                        ps = pp4.tile([P, TQ], FP32, tag="pa")
                        for k in range(8):
                            nc.tensor.matmul(
                                ps[:],
                                wq[k][:, P * f:P * (f + 1)],
                                xt[:, k, :],
                                start=(k == 0), stop=(k == 7),
                            )
                        nc.vector.tensor_copy(qTs[f][:], ps[:])
                    ps = pp4.tile([P, TQ], FP32, tag="pa")
                    for k in range(8):
                        nc.tensor.matmul(ps[:], wk[k][:], xt[:, k, :],
                                         start=(k == 0), stop=(k == 7))
                    nc.vector.tensor_copy(kT[:, ts], ps[:])
                    vT = blk.tile([P, TQ], FP32, tag="vT")
                    ps = pp4.tile([P, TQ], FP32, tag="pa")
                    for k in range(8):
                        nc.tensor.matmul(ps[:], wv[k][:], xt[:, k, :],
                                         start=(k == 0), stop=(k == 7))
                    nc.vector.tensor_copy(vT[:], ps[:])

                    # V to token-major via PE transpose: [64, 128] -> [128, 64]
                    for kv in range(2):
                        ve = vextA if kv == 0 else vextB
                        for jj in range(4):
                            j = 4 * t + jj
                            tp = pp4.tile([P, TQ], FP32, tag="pa")
                            nc.tensor.transpose(
                                tp[:, 0:HD],
                                vT[64 * kv:64 * kv + 64, P * jj:P * (jj + 1)],
                                ident2[64 * kv:64 * kv + 64, :],
                            )
                            nc.vector.tensor_copy(ve[:, j, 0:HD], tp[:, 0:HD])

                    # ---------- stage B(c=t): attention for query block c
                    c = t
                    jmax = 4 * c + 3
                    attnT = [blk.tile([P, TQ], BF16, tag=f"a{p}", name=f"at{p}")
                             for p in range(4)]
                    for p in range(4):
                        # PV accumulators live in the shared 4-buf PSUM pool:
                        # rotation distance 2 pairs, so PV(p+1) never waits on
                        # normalize(p) draining (kept the PE HAM-warm)
                        oaT = pp4.tile([P, TQ], FP32, tag="pa", name="oaT")
                        obT = pp4.tile([P, TQ], FP32, tag="pa", name="obT")
                        outA = oaT[0:HD + 1, :]
                        outB = obT[0:HD + 1, :]
                        for j in range(jmax + 1):
                            r = j - 4 * c
                            # col restriction: cols < 128r are fully masked.
                            # matmul keeps N>=256 (fp32r full-rate floor)
                            moff = 128 * r if r in (1, 2) else 0
                            aoff = 128 * r if r >= 1 else 0
                            s2 = scps.tile([P, 2, TQ], FP32, tag="s2")
                            nc.tensor.matmul(
                                s2[:, 0, moff:],
                                kT[0:64, P * j:P * (j + 1)],
                                qTs[p][0:64, moff:],
                                start=True, stop=True, tile_position=(0, 0),
                            )
                            nc.tensor.matmul(
                                s2[:, 1, moff:],
                                kT[64:128, P * j:P * (j + 1)],
                                qTs[p][64:128, moff:],
                                start=True, stop=True, tile_position=(64, 0),
                            )
                            pt = ptsb.tile([P, 2, TQ], BF16, tag="pt")
                            nc.scalar.activation(pt[:, :, aoff:], s2[:, :, aoff:],
                                                 AF.Exp, scale=0.125)
                            if r >= 0:
                                # causal triangle of the diagonal 128x128
                                # sub-block: keep where tq_local - tk >= 0
                                nc.gpsimd.affine_select(
                                    out=pt[:, :, aoff:aoff + 128],
                                    in_=pt[:, :, aoff:aoff + 128],
                                    compare_op=ALU.is_ge, fill=0.0,
                                    base=0,
                                    pattern=[[0, 2], [1, 128]],
                                    channel_multiplier=-1,
                                )
                            # cols < aoff are fully masked: never computed,
                            # never read -- PV accumulates only [aoff:] (j=0
                            # is always a full block, so PSUM is fully
                            # initialized at start)
                            nc.tensor.matmul(outA[:, aoff:], vextA[:, j, :],
                                             pt[:, 0, aoff:],
                                             start=(j == 0), stop=(j == jmax))
                            nc.tensor.matmul(outB[:, aoff:], vextB[:, j, :],
                                             pt[:, 1, aoff:],
                                             start=(j == 0), stop=(j == jmax))
                        # normalize: recip the denominator row pair, broadcast
                        # over the pair's partitions, scale PV outs into attnT
                        da = nrm.tile([1, TQ], FP32, tag="da")
                        db = nrm.tile([1, TQ], FP32, tag="db")
                        nc.vector.tensor_copy(da[:], outA[64:65, :])
                        nc.vector.tensor_copy(db[:], outB[64:65, :])
                        dar = nrm.tile([1, TQ], FP32, tag="dar")
                        dbr = nrm.tile([1, TQ], FP32, tag="dbr")
                        nc.vector.reciprocal_approx_fast(dar[:], da[:])
                        nc.vector.reciprocal_approx_fast(dbr[:], db[:])
                        bcA = nrm.tile([64, TQ], FP32, tag="bcA")
                        bcB = nrm.tile([64, TQ], FP32, tag="bcB")
                        nc.gpsimd.partition_broadcast(bcA[:], dar[:])
                        nc.gpsimd.partition_broadcast(bcB[:], dbr[:])
                        nc.vector.tensor_mul(attnT[p][0:64, :], outA[0:64, :], bcA[:])
                        nc.vector.tensor_mul(attnT[p][64:128, :], outB[0:64, :], bcB[:])

                    # ---------- stage C(c): output projection rows for block c
                    for tt in range(4 * c, 4 * c + 4):
                        jj = tt - 4 * c
                        yc = scps.tile([P, 2, TQ], FP32, tag="s2")
                        for co in range(2):
                            for p in range(4):
                                nc.tensor.matmul(
                                    yc[:, co, :],
                                    attnT[p][:, P * jj:P * (jj + 1)],
                                    wo[p][:, TQ * co:TQ * (co + 1)],
                                    start=(p == 0), stop=(p == 3),
                                )
                        yt = ptsb.tile([P, 2, TQ], FP32, tag="yt", bufs=2)
                        nc.vector.tensor_copy(yt[:], yc[:])
                        nc.sync.dma_start(out=y[P * tt:P * (tt + 1), :], in_=yt[:])

    nc.compile()
    return nc


def get_program():
    global _PROG
    if _PROG is None:
        _PROG = _build_program()
    return _PROG


def make_in_maps(x, Wq, Wk, Wv, Wo):
    """Build the per-core input dicts (host-side sharding + layout prep)."""
    import ml_dtypes
    bf16 = ml_dtypes.bfloat16
    x = np.asarray(x, np.float32)
    Wq = np.asarray(Wq, np.float32)
    Wk = np.asarray(Wk, np.float32)
    Wv = np.asarray(Wv, np.float32)
    Wo = np.asarray(Wo, np.float32)
    ones = np.ones((P, NTK), bf16)
    in_maps = []
    for core in range(8):
        b, half = core // 2, core % 2
        h0 = 8 * half
        kv0 = 2 * half
        # pair-permuted local head order: [h0, h0+4, h0+1, h0+5, ...]
        heads = []
        for p in range(4):
            heads += [h0 + p, h0 + p + 4]
        qrows = np.concatenate([Wq[h * HD:(h + 1) * HD] for h in heads], 0)  # [512, C]
        wocols = np.concatenate([Wo[:, h * HD:(h + 1) * HD] for h in heads], 1)  # [C, 512]
        in_maps.append({
            "xT": np.ascontiguousarray(x[b].T).astype(bf16),
            "wqT": np.ascontiguousarray(qrows.T).astype(bf16),
            "wkT": np.ascontiguousarray(Wk[kv0 * HD:(kv0 + 2) * HD].T).astype(bf16),
            "wvT": np.ascontiguousarray(Wv[kv0 * HD:(kv0 + 2) * HD].T).astype(bf16),
            "woT": np.ascontiguousarray(wocols.T).astype(bf16),
            "ones": ones,
        })
    return in_maps


def run_on_hw(in_maps, trace=False, **kw):
    from concourse.bass_utils import run_bass_kernel_spmd
    nc = get_program()
    return run_bass_kernel_spmd(nc, in_maps, list(range(8)), trace=trace, **kw)


def kernel(**inputs):
    in_maps = make_in_maps(
        inputs["x"], inputs["Wq"], inputs["Wk"], inputs["Wv"], inputs["Wo"]
    )
    res = run_on_hw(in_maps)
    out = np.empty((4, T, C), np.float32)
    for b in range(4):
        out[b] = res.results[2 * b]["y"] + res.results[2 * b + 1]["y"]
    return out
